# revision 25
# baseline (speedup 1.0000x reference)
"""Causal self-attention (B=4, T=2048, C=1024, H=16, D=64) on 8 trn2 cores.

Sharding: core i handles batch b = i % 4 and head-group g = i // 4
(8 heads per core).  Each core computes QKV for its heads, causal
attention, and a partial projection (w_proj rows for its heads).  The
host sums the two partial projections per batch.

Per-core pipeline, interleaved over Tq chunks c of 512 so the PE-heavy
QKV work of chunk c+1 fills the ACT-bound attention of chunk c:

  for c: QKV(c) -> attention(all heads, Tq chunk c) -> projection(c)

Layouts:
  - X^T built on-chip via PE transposes (f32r).
  - QK^T gemm (f32r) emits Q^T/K^T in [head*64, T] layout, stored bf16.
  - V gemm (f32r) emits V in [T, D] layout, stored bf16 widened to
    [T, 8, 65] with a trailing ones column -> AV matmul row 64 accumulates
    the softmax denominator for free; reciprocal + f32r K=1 broadcast
    matmul + DVE multiply normalize.
  - Scores transposed (S^T [Tk, Tq]) in bf16; causal masking skips dead
    tiles, adds -1e30 to diagonal tiles in PSUM (DVE) before exp.
  - exp on ACT over r-paired [128, 1024] PSUM tiles -> bf16 P^T.
  - Projection in f32r from per-chunk Y^T tiles.
"""
import sys

for _p in ("/opt/trn_rl_repo",):
    if _p not in sys.path:
        sys.path.insert(0, _p)

import numpy as np

import concourse.bass as bass
import concourse.tile as tile
from concourse import bacc, mybir
from concourse.bass_utils import run_bass_kernel_spmd
from concourse.masks import make_identity

F32 = mybir.dt.float32
F32R = mybir.dt.float32r
BF16 = mybir.dt.bfloat16
EXP = mybir.ActivationFunctionType.Exp

B, T, C = 4, 2048, 1024
H, D = 16, 64
HPC = 8              # heads per core
GD = HPC * D         # 512
NCORES = 8
TCH = 512            # Tq / T chunk width
NTCH = T // TCH      # 4
NKT = C // 128       # 8 contraction k-tiles over C
NTT = T // 128       # 16 T tiles

ATTN_DT = BF16       # dtype for Q^T/K^T/V/P^T (scores + AV matmuls)


def _declare_io(nc):
    return dict(
        xb=nc.dram_tensor("xb", [T, C], F32R, kind="ExternalInput").ap(),
        wqk=nc.dram_tensor("wqk", [C, 2 * GD], F32R, kind="ExternalInput").ap(),
        wv=nc.dram_tensor("wv", [C, GD], F32R, kind="ExternalInput").ap(),
        wp=nc.dram_tensor("wp", [GD, C], F32R, kind="ExternalInput").ap(),
        out=nc.dram_tensor("out", [T, C], F32, kind="ExternalOutput").ap(),
    )


def _build_attn(tc, io, rep=""):
    nc = tc.nc
    xb, wqk, wv, wp, out = io["xb"], io["wqk"], io["wv"], io["wp"], io["out"]

    from contextlib import ExitStack
    with ExitStack() as _es:
        constp = _es.enter_context(tc.tile_pool(name=f"const{rep}", bufs=1))
        qktp = _es.enter_context(tc.tile_pool(name=f"qkt{rep}", bufs=1))
        vextp = _es.enter_context(tc.tile_pool(name=f"vext{rep}", bufs=1))
        wqkp = _es.enter_context(tc.tile_pool(name=f"wqk{rep}", bufs=1))
        wvp = _es.enter_context(tc.tile_pool(name=f"wv{rep}", bufs=1))
        wpp = _es.enter_context(tc.tile_pool(name=f"wp{rep}", bufs=1))
        xsbp = _es.enter_context(tc.tile_pool(name=f"xsb{rep}", bufs=4))
        xtp = _es.enter_context(tc.tile_pool(name=f"xt{rep}", bufs=8))
        ytp = _es.enter_context(tc.tile_pool(name=f"yt{rep}", bufs=12))
        ptp = _es.enter_context(tc.tile_pool(name=f"pt{rep}", bufs=8))
        ysbp = _es.enter_context(tc.tile_pool(name=f"ysb{rep}", bufs=3))
        recp = _es.enter_context(tc.tile_pool(name=f"rec{rep}", bufs=2))
        osbp = _es.enter_context(tc.tile_pool(name=f"osb{rep}", bufs=2))
        gps = _es.enter_context(tc.tile_pool(name=f"g_ps{rep}", bufs=1, space="PSUM"))
        sps = _es.enter_context(tc.tile_pool(name=f"s_ps{rep}", bufs=2, space="PSUM"))
        yps = _es.enter_context(tc.tile_pool(name=f"y_ps{rep}", bufs=2, space="PSUM"))
        pjps = _es.enter_context(tc.tile_pool(name=f"pj_ps{rep}", bufs=1, space="PSUM"))

        # ---- constants
        ident = constp.tile([128, 128], F32, tag=f"ident{rep}")
        make_identity(nc, ident[:])
        ident_r = constp.tile([128, 128], F32R, tag=f"ident_r{rep}")
        nc.vector.tensor_copy(ident_r[:], ident[:])
        ones = constp.tile([128, 64], F32, tag=f"ones{rep}")
        nc.gpsimd.memset(ones[:], 1.0)
        ones_r = constp.tile([128, 64], F32R, tag=f"ones_r{rep}")
        nc.vector.tensor_copy(ones_r[:], ones[:])

        # ---- persistent tensors
        qkt = [[qktp.tile([128, TCH], ATTN_DT, tag=f"qkt{m}_{cc}{rep}",
                          name=f"qkt{m}_{cc}{rep}") for cc in range(NTCH)]
               for m in range(8)]
        vext = [vextp.tile([128, HPC * 65], ATTN_DT, tag=f"vext{t}{rep}",
                           name=f"vext{t}{rep}") for t in range(NTT)]

        # ---- weight + first x loads (x chunk 0 first so PE can start)
        x_first = []
        for j in range(4):
            xr = xsbp.tile([128, C], F32R, tag=f"x{rep}", name=f"x0_{j}{rep}")
            x_first.append(xr)
        for half in range(2):
            for j in range(4):
                nc.sync.dma_start(
                    x_first[j][:, bass.ts(half, C // 2)],
                    xb[bass.ds(j * 128, 128), bass.ts(half, C // 2)],
                )
        wqk_sb = [wqkp.tile([128, 2 * GD], F32R, tag=f"wqk{k}{rep}",
                            name=f"wqk{k}{rep}") for k in range(NKT)]
        # load the Q-columns of every k-tile first: the chunk-0 Q^T gemm can
        # then start while the K-columns are still streaming in
        for half in range(2):
            for k in range(NKT):
                nc.scalar.dma_start(
                    wqk_sb[k][:, bass.ts(half, GD)],
                    wqk[bass.ts(k, 128), bass.ts(half, GD)],
                )
        wv_sb = [wvp.tile([128, GD], F32R, tag=f"wv{k}{rep}", name=f"wv{k}{rep}")
                 for k in range(NKT)]
        for k in range(NKT):
            nc.scalar.dma_start(wv_sb[k][:], wv[bass.ts(k, 128), :])
        wp_sb = [wpp.tile([128, C], F32R, tag=f"wp{k}{rep}", name=f"wp{k}{rep}")
                 for k in range(4)]
        for k in range(4):
            nc.scalar.dma_start(wp_sb[k][:], wp[bass.ts(k, 128), :])
        ones8 = ones[:, 0:8].rearrange("p (h e) -> p h e", e=1)
        for t in range(NTT):
            nc.vector.tensor_copy(
                vext[t][:].rearrange("p (h e) -> p h e", e=65)[:, :, 64:65], ones8
            )

        # ---- main pipeline.  Per-engine execution follows emission order,
        # so QKV(c+1) / deferred projections are emitted as "filler" units
        # interleaved between attention steps of chunk c to keep the PE busy
        # while ACT works through the exps.
        from collections import deque

        def emit_x_dma(c):
            x_sb = []
            for j in range(4):
                xr = xsbp.tile([128, C], F32R, tag=f"x{rep}", name=f"x{c}_{j}{rep}")
                nc.sync.dma_start(xr[:], xb[bass.ds(c * TCH + j * 128, 128), :])
                x_sb.append(xr)
            return x_sb

        def emit_tp(c, ct, x_sb, xt_sb):
            # chunk 0 runs before attention: borrow the idle yext slots so
            # transposes/copies double-buffer despite the single g slot
            pool, tag = (yps, f"yext{rep}") if c == 0 else (gps, f"g{rep}")
            t_ps = pool.tile([128, TCH], F32R, tag=tag, name=f"tp{c}_{ct}{rep}")
            for j in range(4):
                nc.tensor.transpose(
                    t_ps[:, bass.ts(j, 128)],
                    x_sb[j][:, bass.ts(ct, 128)],
                    ident_r[:],
                )
            xt_t = xtp.tile([128, TCH], F32R, tag=f"xt{rep}", name=f"xt{c}_{ct}{rep}")
            nc.vector.tensor_copy(xt_t[:], t_ps[:])
            xt_sb.append(xt_t)

        def emit_qk_mtile(c, m, xt_sb):
            if c == 0:
                pool, tag = (pjps, f"pj{rep}") if m % 2 else (yps, f"yext{rep}")
            else:
                pool, tag = gps, f"g{rep}"
            o_ps = pool.tile([128, TCH], F32, tag=tag, name=f"qk{c}_{m}{rep}")
            for k in range(NKT):
                nc.tensor.matmul(
                    out=o_ps[:],
                    lhsT=wqk_sb[k][:, bass.ts(m, 128)],
                    rhs=xt_sb[k][:],
                    start=(k == 0),
                    stop=(k == NKT - 1),
                )
            nc.vector.tensor_copy(qkt[m][c][:], o_ps[:])

        def emit_v_jtile(c, j, xt_sb):
            if c == 0:
                pool, tag = (pjps, f"pj{rep}") if j % 2 else (yps, f"yext{rep}")
            else:
                pool, tag = gps, f"g{rep}"
            o_ps = pool.tile([128, GD], F32, tag=tag, name=f"v{c}_{j}{rep}")
            for k in range(NKT):
                nc.tensor.matmul(
                    out=o_ps[:],
                    lhsT=xt_sb[k][:, bass.ts(j, 128)],
                    rhs=wv_sb[k][:],
                    start=(k == 0),
                    stop=(k == NKT - 1),
                )
            dst = vext[c * 4 + j][:].rearrange("p (h e) -> p h e", e=65)
            nc.vector.tensor_copy(
                dst[:, :, 0:64],
                o_ps[:].rearrange("p (h e) -> p h e", e=64),
            )

        def emit_proj_group(c, tt, n, yt_c, alt=False, osb_acc={}):
            t = 4 * c + tt
            if alt and (tt * 2 + n) % 2:
                pool, tag = gps, f"g{rep}"
            else:
                pool, tag = pjps, f"pj{rep}"
            o_ps = pool.tile([128, 512], F32, tag=tag, name=f"pj{t}_{n}{rep}")
            for k in range(4):
                nc.tensor.matmul(
                    out=o_ps[:],
                    lhsT=yt_c[k][:, bass.ts(tt, 128)],
                    rhs=wp_sb[k][:, bass.ts(n, 512)],
                    start=(k == 0),
                    stop=(k == 3),
                )
            # pair the two half-rows into one osb tile and one 512KB store,
            # alternating DMA queues so the tail is not single-queue paced
            if n == 0:
                osb_acc[t] = osbp.tile([128, C], F32, tag=f"osb{rep}", name=f"osb{t}{rep}")
            osb = osb_acc[t]
            nc.vector.tensor_copy(osb[:, bass.ts(n, 512)], o_ps[:])
            if n == 1:
                eng = nc.sync if t % 2 else nc.scalar
                eng.dma_start(out[bass.ts(t, 128), :], osb[:])
                del osb_acc[t]

        class Pacer:
            """Spreads filler units across a chunk's attention ticks.

            Pops are biased to the post-scores tick (between exp emission and
            the AV matmuls that consume it) where the PE would otherwise wait
            on the ACT engine.
            """

            def __init__(self, filler, total_ticks, reserve=0):
                self.filler = filler
                self.total = max(total_ticks, 1)
                self.supply = max(len(filler) - reserve, 0)
                self.tick = 0
                self.pops = 0

            def _pop_to(self, target):
                while self.filler and self.pops < target:
                    self.filler.popleft()()
                    self.pops += 1

            def tick_pts(self):
                self.tick += 1
                self._pop_to(-(-self.supply * self.tick // self.total))  # ceil

            def tick_av(self):
                self.tick += 1
                self._pop_to(self.supply * self.tick // self.total)  # floor

            def drain(self):
                while self.filler:
                    self.filler.popleft()()

        def emit_att_pair(c, hp, yt_c, pacer, lag=2):
            nr = 4 * c + 4
            heads = (2 * hp, 2 * hp + 1)
            qtile = qkt[hp]
            ktile = qkt[4 + hp]
            yext = {h: yps.tile([128, TCH], F32, tag=f"yext{rep}",
                                name=f"yext{h}_{c}{rep}") for h in heads}

            def emit_scores(r0):
                pts = {}
                for h in heads:
                    pr = 64 * (h % 2)
                    s_ps = sps.tile([128, 2 * TCH], F32, tag=f"s{rep}")
                    for rr in (r0, r0 + 1):
                        nc.tensor.matmul(
                            out=s_ps[:, bass.ts(rr - r0, TCH)],
                            lhsT=ktile[rr // 4][pr:pr + 64, bass.ts(rr % 4, 128)],
                            rhs=qtile[c][pr:pr + 64, :],
                            start=True,
                            stop=True,
                        )
                    pt = ptp.tile([128, 2 * TCH], ATTN_DT, tag=f"pt{rep}",
                                  name=f"pt{c}_{h}_{r0}{rep}")
                    nc.scalar.activation(pt[:], s_ps[:], EXP, scale=0.125)
                    for rr in (r0, r0 + 1):
                        j = rr - 4 * c
                        if 0 <= j <= 3:
                            w = 128 * (j + 1)
                            off = (rr - r0) * TCH
                            nc.gpsimd.affine_select(
                                out=pt[:, bass.ds(off, w)],
                                in_=pt[:, bass.ds(off, w)],
                                compare_op=mybir.AluOpType.is_ge,
                                fill=0.0,
                                base=-128 * j,
                                pattern=[[1, w]],
                                channel_multiplier=-1,
                            )
                    pts[h] = pt
                return pts

            def emit_av(r0, pts):
                for h in heads:
                    for rr in (r0, r0 + 1):
                        nc.tensor.matmul(
                            out=yext[h][0:65, :],
                            lhsT=vext[rr][:, h * 65:h * 65 + 65],
                            rhs=pts[h][:, bass.ts(rr - r0, TCH)],
                            start=(rr == 0),
                            stop=(rr == nr - 1),
                        )

            # software pipeline: AV for pair p-lag runs while scores for pair
            # p are computed, so the AV matmuls never wait on a fresh exp.
            # Scores are emitted two pairs back-to-back so the PE runs longer
            # uninterrupted stretches in the 64-row tile mode.
            pend = deque()
            for r0 in range(0, nr, 4):
                pend.append((r0, emit_scores(r0)))
                if r0 + 2 < nr:
                    pend.append((r0 + 2, emit_scores(r0 + 2)))
                pacer.tick_pts()
                pacer.tick_pts()
                while len(pend) > lag:
                    emit_av(*pend.popleft())
                pacer.tick_av()
                pacer.tick_av()
            while pend:
                emit_av(*pend.popleft())
            for h in heads:
                pr = 64 * (h % 2)
                ysb = ysbp.tile([128, TCH], F32, tag=f"ysb{rep}", name=f"ysb{h}_{c}{rep}")
                nc.vector.tensor_copy(ysb[0:65, :], yext[h][0:65, :])
                rec = recp.tile([128, TCH], F32R, tag=f"rec{rep}")
                with nc.allow_low_precision(reason="f32r denominators"):
                    nc.vector.reciprocal(rec[64:65, :], ysb[64:65, :])
                bc = pjps.tile([64, TCH], F32, tag=f"pj{rep}", name=f"bc{h}_{c}{rep}")
                nc.tensor.matmul(
                    out=bc[:],
                    lhsT=ones_r[64:65, :],
                    rhs=rec[64:65, :],
                    start=True,
                    stop=True,
                )
                nc.vector.tensor_mul(
                    yt_c[hp][pr:pr + 64, :],
                    ysb[0:64, :],
                    bc[:],
                )

        # QKV(0) emitted directly (nothing to interleave into)
        xt_store = {}
        xt_store[0] = []
        for ct in range(NKT):
            emit_tp(0, ct, x_first, xt_store[0])
        for m in range(8):
            emit_qk_mtile(0, m, xt_store[0])
        for j in range(4):
            emit_v_jtile(0, j, xt_store[0])

        yt_store = {}
        for c in range(NTCH):
            yt_c = [ytp.tile([128, TCH], F32R, tag=f"yt{rep}", name=f"yt{c}_{k}{rep}")
                    for k in range(4)]
            yt_store[c] = yt_c
            filler = deque()
            if c < 3:
                cn = c + 1
                x_next = emit_x_dma(cn)
                xt_store[cn] = []
                for ct in range(NKT):
                    filler.append(lambda cn=cn, ct=ct, xn=x_next: emit_tp(cn, ct, xn, xt_store[cn]))
                mlist = range(8) if cn < 3 else range(5)   # chunk 3: Q part + K m4
                for m in mlist:
                    filler.append(lambda cn=cn, m=m: emit_qk_mtile(cn, m, xt_store[cn]))
                for j in range(4):
                    filler.append(lambda cn=cn, j=j: emit_v_jtile(cn, j, xt_store[cn]))
                if c == 2:
                    # proj(0) deadline: yt(0) buffers are recycled by yt(3)
                    for tt in range(4):
                        for n in range(2):
                            filler.append(lambda tt=tt, n=n: emit_proj_group(0, tt, n, yt_store[0]))
            else:
                for tt in range(4):
                    for n in range(2):
                        filler.append(lambda tt=tt, n=n: emit_proj_group(1, tt, n, yt_store[1]))
                        filler.append(lambda tt=tt, n=n: emit_proj_group(2, tt, n, yt_store[2]))
            pacer = Pacer(filler, total_ticks=(HPC // 2) * (2 * c + 2) * 2)
            for hp in range(HPC // 2):
                if c == 3 and hp > 0:   # K^T tile for this head pair
                    emit_qk_mtile(3, 4 + hp, xt_store[3])
                emit_att_pair(c, hp, yt_c, pacer)
            pacer.drain()
        for tt in range(4):
            for n in range(2):
                emit_proj_group(3, tt, n, yt_store[3], alt=True)


_NC_CACHE = None


def _get_nc(reps=1, loop=0):
    """reps: unrolled body copies; loop: hardware For_i wrap (timing only)."""
    global _NC_CACHE
    key = (reps, loop)
    if _NC_CACHE is None or _NC_CACHE[0] != key:
        nc = bacc.Bacc("TRN2", target_bir_lowering=False, debug=False,
                       num_devices=NCORES)
        with tile.TileContext(nc, trace_sim=False) as tc:
            io = _declare_io(nc)
            if loop:
                with tc.For_i(0, loop, 1):
                    _build_attn(tc, io)
            else:
                for r in range(reps):
                    _build_attn(tc, io, rep="" if reps == 1 else f"_r{r}")
        nc.compile()
        _NC_CACHE = (key, nc)
    return _NC_CACHE[1]


def shard_inputs(x, w_qkv, w_proj):
    """Build the 8 per-core input maps."""
    in_maps = []
    for i in range(NCORES):
        b, g = i % B, i // B
        cols = slice(g * GD, (g + 1) * GD)
        in_maps.append({
            "xb": np.ascontiguousarray(x[b]),
            "wqk": np.ascontiguousarray(
                np.concatenate([w_qkv[:, 0 * C:][:, cols], w_qkv[:, 1 * C:][:, cols]], axis=1)
            ),
            "wv": np.ascontiguousarray(w_qkv[:, 2 * C:][:, cols]),
            "wp": np.ascontiguousarray(w_proj[g * GD:(g + 1) * GD, :]),
        })
    return in_maps


def unshard_output(results):
    out = np.empty((B, T, C), dtype=np.float32)
    for b in range(B):
        out[b] = results[b]["out"] + results[b + B]["out"]
    return out


def kernel(x, w_qkv, w_proj):
    x = np.asarray(x, dtype=np.float32)
    w_qkv = np.asarray(w_qkv, dtype=np.float32)
    w_proj = np.asarray(w_proj, dtype=np.float32)
    nc = _get_nc()
    in_maps = shard_inputs(x, w_qkv, w_proj)
    res = run_bass_kernel_spmd(nc, in_maps, list(range(NCORES)))
    return unshard_output(res.results)

